# revision 3
# baseline (speedup 1.0000x reference)
"""Trainium2 Bass kernel for nn_MixingBlock (slot-attention mixing block).

Math (per batch b):
    xn   = layernorm(x, g1, b1)          # (I, 180)
    q    = xn @ Wq                       # (I, 1536)
    k    = slots @ Wk                    # (64, 1536)
    dots = q @ k.T / sqrt(1536)          # (I, 64)
    w    = softmax(dots, axis=-1)        # (I, 64)   [output]
    s    = layernorm(w @ slots, g2, b2)  # (I, 1536) [output]

Sharding: pure data-parallel over B (B == 8 == n_cores); each core owns one
batch and does no cross-core communication.

Per-core orientation plan (contraction must live on the partition axis):
  - LN1 in natural layout, then PE-transpose xn tiles -> xnT (180 on
    partitions, split 128+52).
  - q^T = Wq_block.T @ xnT  (per 128-wide att block; Wq used in natural
    layout as the stationary operand).
  - dots^T = kT_sub.T-accumulated over 12 att subtiles @ q^T (k^T built
    once at startup from slots^T and Wk).
  - exp without max-subtraction (dots are O(+-6), safe in fp32); the
    1/sqrt(1536) scale is folded into the ACT Exp's input scale.
  - natural-layout w tiles via PE-transpose of exp^T; row sums + normalize
    give the w output.
  - s_un = exp^T.T @ slots with *unnormalized* attention; the softmax
    normalization is folded into LN2's per-row scalars exactly:
      out = (s - mu_s) * rsqrt(var_s + eps) * g2 + b2,  s = r * s_un
          = (s_un - mu_un) * [r * rsqrt(r^2 var_un + eps)] * g2 + b2
    where r = 1/rowsum(exp).
"""

import os
from contextlib import ExitStack

import numpy as np

INPUT_DIM, SLOT_DIM, ATT_DIM = 180, 1536, 1536
B, I_FULL, S = 8, 8192, 64
N_CORES = 8
K0 = 128
K1 = INPUT_DIM - K0  # 52
CH = 512             # rows per chunk
P = 128
TPC = CH // P        # row-tiles per chunk
NA = ATT_DIM // P    # 12 att-dim subtiles
NN = SLOT_DIM // CH  # 3 N-chunks of the 1536-wide outputs
NK = SLOT_DIM // P   # 12 slot-dim subtiles
SCALE = float(ATT_DIM) ** -0.5
EPS = 1e-5


def emit(tc, ctx, x, slots, Wq, Wk, g1, b1, g2, b2, out_s, out_w, I_rows):
    """Emit the Tile program. All tensor args are bass APs."""
    import concourse.bass as bass
    import concourse.mybir as mybir
    from concourse.bass import ts
    from concourse.masks import make_identity

    nc = tc.nc
    fp32 = mybir.dt.float32
    AF = mybir.ActivationFunctionType
    OP = mybir.AluOpType
    AX = mybir.AxisListType

    n_chunks = I_rows // CH
    assert I_rows % CH == 0

    const = ctx.enter_context(tc.tile_pool(name="const", bufs=1))
    wkpool = ctx.enter_context(tc.tile_pool(name="wkstream", bufs=2))
    xpool = ctx.enter_context(tc.tile_pool(name="xin", bufs=3))
    xnpool = ctx.enter_context(tc.tile_pool(name="xn", bufs=3))
    stpool = ctx.enter_context(tc.tile_pool(name="stats", bufs=6))
    xntpool = ctx.enter_context(tc.tile_pool(name="xnt", bufs=2))
    qtpool = ctx.enter_context(tc.tile_pool(name="qt", bufs=2))
    etpool = ctx.enter_context(tc.tile_pool(name="expT", bufs=2))
    wnpool = ctx.enter_context(tc.tile_pool(name="wnat", bufs=3))
    outpool = ctx.enter_context(tc.tile_pool(name="outs", bufs=3))

    # PSUM pools — 8 banks total: 2 + 3 + 1 + 2.
    qpsum = ctx.enter_context(tc.tile_pool(name="qpsum", bufs=2, space="PSUM"))
    spsum = ctx.enter_context(tc.tile_pool(name="spsum", bufs=3, space="PSUM"))
    dpsum = ctx.enter_context(tc.tile_pool(name="dpsum", bufs=1, space="PSUM"))
    smallp = ctx.enter_context(tc.tile_pool(name="smallpsum", bufs=2, space="PSUM"))

    # ---- constants -------------------------------------------------------
    ident = const.tile([P, P], fp32)
    make_identity(nc, ident)

    wq0 = const.tile([K0, ATT_DIM], fp32)
    nc.sync.dma_start(out=wq0, in_=Wq[0:K0, :])
    wq1 = const.tile([K1, ATT_DIM], fp32)
    nc.sync.dma_start(out=wq1, in_=Wq[K0:INPUT_DIM, :])

    slots_sb = const.tile([S, SLOT_DIM], fp32)
    nc.sync.dma_start(out=slots_sb, in_=slots)

    def bcast(ap):
        return bass.AP(tensor=ap.tensor, offset=ap.offset, ap=[[0, P], *ap.ap])

    g1r = const.tile([P, INPUT_DIM], fp32)
    nc.gpsimd.dma_start(out=g1r, in_=bcast(g1))
    b1r = const.tile([P, INPUT_DIM], fp32)
    nc.gpsimd.dma_start(out=b1r, in_=bcast(b1))
    g2r = const.tile([P, SLOT_DIM], fp32)
    nc.gpsimd.dma_start(out=g2r, in_=bcast(g2))
    b2r = const.tile([P, SLOT_DIM], fp32)
    nc.gpsimd.dma_start(out=b2r, in_=bcast(b2))

    eps_t = const.tile([P, 1], fp32)
    nc.vector.memset(eps_t, EPS)

    # ---- slots^T: [64, 1536] -> 12 x [128, 64] ---------------------------
    slotsT = const.tile([P, NK, S], fp32)
    for j in range(NK):
        pt = smallp.tile([P, P], fp32, tag="smallp")
        nc.tensor.transpose(pt[:, :S], slots_sb[:, ts(j, P)], ident[:S, :S])
        nc.any.tensor_copy(out=slotsT[:, j, :], in_=pt[:, :S])

    # ---- k^T = (slots @ Wk)^T : 12 x [128, 64] ---------------------------
    # kT[a, s] = sum_d Wk[d, a] * slotsT[d, s]
    kT = const.tile([P, NA, S], fp32)
    for j in range(NA):
        wkblk = wkpool.tile([P, NK, P], fp32, tag="wkblk")
        nc.sync.dma_start(
            out=wkblk,
            in_=Wk.rearrange("(ko ki) a -> ki ko a", ki=P)[:, :, ts(j, P)],
        )
        pk = smallp.tile([P, P], fp32, tag="smallp")
        for ksub in range(NK):
            nc.tensor.matmul(
                pk[:, :S],
                wkblk[:, ksub, :],
                slotsT[:, ksub, :],
                start=(ksub == 0),
                stop=(ksub == NK - 1),
            )
        nc.any.tensor_copy(out=kT[:, j, :], in_=pk[:, :S])

    # ---- main loop over row chunks ---------------------------------------
    for c in range(n_chunks):
        r0 = c * CH

        # LN1 + transpose -> xnT (180 x CH, split 128 + 52)
        xnT0 = xntpool.tile([P, CH], fp32, tag="xnt0")
        xnT1 = xntpool.tile([K1, CH], fp32, tag="xnt1")
        for t in range(TPC):
            xt = xpool.tile([P, INPUT_DIM], fp32, tag="xt")
            nc.sync.dma_start(out=xt, in_=x[r0 + t * P : r0 + (t + 1) * P, :])

            st1 = stpool.tile([P, 6], fp32, tag="st1")
            nc.vector.bn_stats(out=st1, in_=xt)
            mv1 = stpool.tile([P, 2], fp32, tag="mv1")
            nc.vector.bn_aggr(out=mv1, in_=st1)

            rstd1 = stpool.tile([P, 1], fp32, tag="rstd1")
            nc.scalar.activation(
                out=rstd1, in_=mv1[:, 1:2], func=AF.Sqrt, bias=eps_t, scale=1.0
            )
            nc.vector.reciprocal(out=rstd1, in_=rstd1)

            xn = xnpool.tile([P, INPUT_DIM], fp32, tag="xn")
            nc.vector.tensor_scalar(
                out=xn,
                in0=xt,
                scalar1=mv1[:, 0:1],
                scalar2=rstd1,
                op0=OP.subtract,
                op1=OP.mult,
            )
            nc.any.tensor_mul(out=xn, in0=xn, in1=g1r)
            nc.any.tensor_add(out=xn, in0=xn, in1=b1r)

            tp0 = smallp.tile([P, P], fp32, tag="smallp")
            nc.tensor.transpose(tp0, xn[:, 0:K0], ident)
            nc.any.tensor_copy(out=xnT0[:, ts(t, P)], in_=tp0)
            tp1 = smallp.tile([P, P], fp32, tag="smallp")
            nc.tensor.transpose(tp1[:K1, :], xn[:, K0:INPUT_DIM], ident)
            nc.any.tensor_copy(out=xnT1[:, ts(t, P)], in_=tp1[:K1, :])

        # q^T chunk: [1536, CH] as 12 x [128, CH]
        qT = qtpool.tile([P, NA, CH], fp32, tag="qt")
        for j in range(NA):
            pq = qpsum.tile([P, CH], fp32, tag="qpsum")
            nc.tensor.matmul(pq, wq0[:, ts(j, P)], xnT0, start=True, stop=False)
            nc.tensor.matmul(pq, wq1[:, ts(j, P)], xnT1, start=False, stop=True)
            nc.any.tensor_copy(out=qT[:, j, :], in_=pq)

        # dots^T chunk: [64, CH]
        pd = dpsum.tile([S, CH], fp32, tag="dpsum")
        for j in range(NA):
            nc.tensor.matmul(
                pd, kT[:, j, :], qT[:, j, :], start=(j == 0), stop=(j == NA - 1)
            )

        # exp (scale folded in); unnormalized attention weights, transposed
        expT = etpool.tile([S, CH], fp32, tag="expT")
        nc.scalar.activation(out=expT, in_=pd, func=AF.Exp, scale=SCALE)

        for t in range(TPC):
            rows = slice(r0 + t * P, r0 + (t + 1) * P)

            # natural-layout w tile + row sums -> normalized w output
            pw = smallp.tile([P, P], fp32, tag="smallp")
            nc.tensor.transpose(pw[:, :S], expT[:, ts(t, P)], ident[:S, :S])
            rsum = stpool.tile([P, 1], fp32, tag="rsum")
            nc.vector.reduce_sum(out=rsum, in_=pw[:, :S], axis=AX.X)
            rcp = stpool.tile([P, 1], fp32, tag="rcp")
            nc.vector.reciprocal(out=rcp, in_=rsum)
            wn = wnpool.tile([P, S], fp32, tag="wn")
            nc.vector.tensor_scalar_mul(wn, pw[:, :S], rcp)
            nc.sync.dma_start(out=out_w[rows, :], in_=wn)

            # s_un = expT_t.T @ slots  (3 x [128, 512] psum tiles)
            ps_list = []
            for nj in range(NN):
                ps = spsum.tile([P, CH], fp32, tag="spsum")
                nc.tensor.matmul(
                    ps,
                    expT[:, ts(t, P)],
                    slots_sb[:, ts(nj, CH)],
                    start=True,
                    stop=True,
                )
                ps_list.append(ps)

            # LN2 stats on unnormalized s
            st2 = stpool.tile([P, NN, 6], fp32, tag="st2")
            for nj in range(NN):
                nc.vector.bn_stats(out=st2[:, nj, :], in_=ps_list[nj])
            mv2 = stpool.tile([P, 2], fp32, tag="mv2")
            nc.vector.bn_aggr(out=mv2, in_=st2)

            # c = rcp * rsqrt(rcp^2 * var_un + eps)  (exact softmax folding)
            rcp2 = stpool.tile([P, 1], fp32, tag="rcp2")
            nc.vector.tensor_mul(out=rcp2, in0=rcp, in1=rcp)
            cmul = stpool.tile([P, 1], fp32, tag="cmul")
            nc.vector.tensor_mul(out=cmul, in0=mv2[:, 1:2], in1=rcp2)
            nc.scalar.activation(
                out=cmul, in_=cmul, func=AF.Sqrt, bias=eps_t, scale=1.0
            )
            nc.vector.reciprocal(out=cmul, in_=cmul)
            nc.vector.tensor_mul(out=cmul, in0=cmul, in1=rcp)

            ot = outpool.tile([P, SLOT_DIM], fp32, tag="ot")
            for nj in range(NN):
                nc.vector.tensor_scalar(
                    out=ot[:, ts(nj, CH)],
                    in0=ps_list[nj],
                    scalar1=mv2[:, 0:1],
                    scalar2=cmul,
                    op0=OP.subtract,
                    op1=OP.mult,
                )
            nc.gpsimd.tensor_mul(out=ot, in0=ot, in1=g2r)
            nc.gpsimd.tensor_add(out=ot, in0=ot, in1=b2r)
            nc.sync.dma_start(out=out_s[rows, :], in_=ot)


def build(I_rows=I_FULL):
    import concourse.bacc as bacc
    import concourse.mybir as mybir
    import concourse.tile as tile

    fp32 = mybir.dt.float32
    nc = bacc.Bacc(
        "TRN2",
        target_bir_lowering=False,
        debug=False,
        enable_asserts=False,
        num_devices=N_CORES,
    )
    aps = {}
    aps["x"] = nc.dram_tensor("x", [I_rows, INPUT_DIM], fp32, kind="ExternalInput").ap()
    aps["slots"] = nc.dram_tensor("slots", [S, SLOT_DIM], fp32, kind="ExternalInput").ap()
    aps["Wq"] = nc.dram_tensor("Wq", [INPUT_DIM, ATT_DIM], fp32, kind="ExternalInput").ap()
    aps["Wk"] = nc.dram_tensor("Wk", [SLOT_DIM, ATT_DIM], fp32, kind="ExternalInput").ap()
    for n, d in (("g1", INPUT_DIM), ("b1", INPUT_DIM), ("g2", SLOT_DIM), ("b2", SLOT_DIM)):
        aps[n] = nc.dram_tensor(n, [d], fp32, kind="ExternalInput").ap()
    aps["out_s"] = nc.dram_tensor(
        "out_s", [I_rows, SLOT_DIM], fp32, kind="ExternalOutput"
    ).ap()
    aps["out_w"] = nc.dram_tensor("out_w", [I_rows, S], fp32, kind="ExternalOutput").ap()

    with tile.TileContext(nc) as tc, ExitStack() as ctx:
        emit(
            tc,
            ctx,
            aps["x"],
            aps["slots"],
            aps["Wq"],
            aps["Wk"],
            aps["g1"],
            aps["b1"],
            aps["g2"],
            aps["b2"],
            aps["out_s"],
            aps["out_w"],
            I_rows,
        )
    nc.compile()
    return nc


_NC_CACHE = {}


def kernel(**inputs):
    from concourse.bass_utils import run_bass_kernel_spmd

    x = np.ascontiguousarray(inputs["x"], dtype=np.float32)
    slots = np.ascontiguousarray(inputs["slot_latents"], dtype=np.float32)
    Wq = np.ascontiguousarray(inputs["Wq"], dtype=np.float32)
    Wk = np.ascontiguousarray(inputs["Wk"], dtype=np.float32)
    g1 = np.ascontiguousarray(inputs["g1"], dtype=np.float32)
    b1 = np.ascontiguousarray(inputs["b1"], dtype=np.float32)
    g2 = np.ascontiguousarray(inputs["g2"], dtype=np.float32)
    b2 = np.ascontiguousarray(inputs["b2"], dtype=np.float32)

    bsz, I_rows, _ = x.shape
    assert bsz == N_CORES

    if I_rows not in _NC_CACHE:
        _NC_CACHE[I_rows] = build(I_rows)
    nc = _NC_CACHE[I_rows]

    in_maps = [
        {
            "x": x[b],
            "slots": slots[b],
            "Wq": Wq,
            "Wk": Wk,
            "g1": g1,
            "b1": b1,
            "g2": g2,
            "b2": b2,
        }
        for b in range(N_CORES)
    ]
    res = run_bass_kernel_spmd(nc, in_maps, core_ids=list(range(N_CORES)))
    s = np.stack([r["out_s"] for r in res.results])
    w = np.stack([r["out_w"] for r in res.results])
    return s, w


# revision 5
# speedup vs baseline: 9796.3489x; 9796.3489x over previous
"""Trainium2 Bass kernel for nn_MixingBlock (slot-attention mixing block).

Math (per batch b):
    xn   = layernorm(x, g1, b1)          # (I, 180)
    q    = xn @ Wq                       # (I, 1536)
    k    = slots @ Wk                    # (64, 1536)
    dots = q @ k.T / sqrt(1536)          # (I, 64)
    w    = softmax(dots, axis=-1)        # (I, 64)   [output]
    s    = layernorm(w @ slots, g2, b2)  # (I, 1536) [output]

Sharding: pure data-parallel over B (B == 8 == n_cores); each core owns one
batch and does no cross-core communication.

Per-core orientation plan (contraction must live on the partition axis):
  - LN1 in natural layout, then PE-transpose xn tiles -> xnT (180 on
    partitions, split 128+52).
  - q^T = Wq_block.T @ xnT  (per 128-wide att block; Wq used in natural
    layout as the stationary operand).
  - dots^T = kT_sub.T-accumulated over 12 att subtiles @ q^T (k^T built
    once at startup from slots^T and Wk).
  - exp without max-subtraction (dots are O(+-6), safe in fp32); the
    1/sqrt(1536) scale is folded into the ACT Exp's input scale.
  - natural-layout w tiles via PE-transpose of exp^T; row sums + normalize
    give the w output.
  - s_un = exp^T.T @ slots with *unnormalized* attention; the softmax
    normalization is folded into LN2's per-row scalars exactly:
      out = (s - mu_s) * rsqrt(var_s + eps) * g2 + b2,  s = r * s_un
          = (s_un - mu_un) * [r * rsqrt(r^2 var_un + eps)] * g2 + b2
    where r = 1/rowsum(exp).
"""

import os
from contextlib import ExitStack

import numpy as np

INPUT_DIM, SLOT_DIM, ATT_DIM = 180, 1536, 1536
B, I_FULL, S = 8, 8192, 64
N_CORES = 8
K0 = 128
K1 = INPUT_DIM - K0  # 52
CH = 512             # rows per chunk
P = 128
TPC = CH // P        # row-tiles per chunk
NA = ATT_DIM // P    # 12 att-dim subtiles
NN = SLOT_DIM // CH  # 3 N-chunks of the 1536-wide outputs
NK = SLOT_DIM // P   # 12 slot-dim subtiles
SCALE = float(ATT_DIM) ** -0.5
EPS = 1e-5


def emit(tc, ctx, x, slots, Wq, Wk, g1, b1, g2, b2, out_s, out_w, I_rows):
    """Emit the Tile program. All tensor args are bass APs."""
    import concourse.bass as bass
    import concourse.mybir as mybir
    from concourse.bass import ts
    from concourse.masks import make_identity

    nc = tc.nc
    fp32 = mybir.dt.float32
    AF = mybir.ActivationFunctionType
    OP = mybir.AluOpType
    AX = mybir.AxisListType

    n_chunks = I_rows // CH
    assert I_rows % CH == 0

    const = ctx.enter_context(tc.tile_pool(name="const", bufs=1))
    wkpool = ctx.enter_context(tc.tile_pool(name="wkstream", bufs=2))
    xpool = ctx.enter_context(tc.tile_pool(name="xin", bufs=3))
    xnpool = ctx.enter_context(tc.tile_pool(name="xn", bufs=3))
    stpool = ctx.enter_context(tc.tile_pool(name="stats", bufs=6))
    xntpool = ctx.enter_context(tc.tile_pool(name="xnt", bufs=2))
    qtpool = ctx.enter_context(tc.tile_pool(name="qt", bufs=2))
    etpool = ctx.enter_context(tc.tile_pool(name="expT", bufs=2))
    wnpool = ctx.enter_context(tc.tile_pool(name="wnat", bufs=3))
    outpool = ctx.enter_context(tc.tile_pool(name="outs", bufs=3))

    # PSUM pools — 8 banks total: 2 + 3 + 1 + 2.
    qpsum = ctx.enter_context(tc.tile_pool(name="qpsum", bufs=2, space="PSUM"))
    spsum = ctx.enter_context(tc.tile_pool(name="spsum", bufs=3, space="PSUM"))
    dpsum = ctx.enter_context(tc.tile_pool(name="dpsum", bufs=1, space="PSUM"))
    smallp = ctx.enter_context(tc.tile_pool(name="smallpsum", bufs=2, space="PSUM"))

    # ---- constants -------------------------------------------------------
    ident = const.tile([P, P], fp32)
    make_identity(nc, ident)

    wq0 = const.tile([K0, ATT_DIM], fp32)
    nc.sync.dma_start(out=wq0, in_=Wq[0:K0, :])
    wq1 = const.tile([K1, ATT_DIM], fp32)
    nc.sync.dma_start(out=wq1, in_=Wq[K0:INPUT_DIM, :])

    slots_sb = const.tile([S, SLOT_DIM], fp32)
    nc.sync.dma_start(out=slots_sb, in_=slots)

    def bcast(ap):
        return bass.AP(tensor=ap.tensor, offset=ap.offset, ap=[[0, P], *ap.ap])

    g1r = const.tile([P, INPUT_DIM], fp32)
    nc.gpsimd.dma_start(out=g1r, in_=bcast(g1))
    b1r = const.tile([P, INPUT_DIM], fp32)
    nc.gpsimd.dma_start(out=b1r, in_=bcast(b1))
    g2r = const.tile([P, SLOT_DIM], fp32)
    nc.gpsimd.dma_start(out=g2r, in_=bcast(g2))
    b2r = const.tile([P, SLOT_DIM], fp32)
    nc.gpsimd.dma_start(out=b2r, in_=bcast(b2))

    eps_t = const.tile([P, 1], fp32)
    nc.vector.memset(eps_t, EPS)

    # ---- slots^T: [64, 1536] -> 12 x [128, 64] ---------------------------
    slotsT = const.tile([P, NK, S], fp32)
    for j in range(NK):
        pt = smallp.tile([P, P], fp32, tag="smallp")
        nc.tensor.transpose(pt[:, :S], slots_sb[:, ts(j, P)], ident[:S, :S])
        nc.any.tensor_copy(out=slotsT[:, j, :], in_=pt[:, :S])

    # ---- k^T = (slots @ Wk)^T : 12 x [128, 64] ---------------------------
    # kT[a, s] = sum_d Wk[d, a] * slotsT[d, s]
    kT = const.tile([P, NA, S], fp32)
    for j in range(NA):
        wkblk = wkpool.tile([P, NK, P], fp32, tag="wkblk")
        nc.sync.dma_start(
            out=wkblk,
            in_=Wk.rearrange("(ko ki) a -> ki ko a", ki=P)[:, :, ts(j, P)],
        )
        pk = smallp.tile([P, P], fp32, tag="smallp")
        for ksub in range(NK):
            nc.tensor.matmul(
                pk[:, :S],
                wkblk[:, ksub, :],
                slotsT[:, ksub, :],
                start=(ksub == 0),
                stop=(ksub == NK - 1),
            )
        nc.any.tensor_copy(out=kT[:, j, :], in_=pk[:, :S])

    # ---- main loop over row chunks ---------------------------------------
    for c in range(n_chunks):
        r0 = c * CH

        # LN1 + transpose -> xnT (180 x CH, split 128 + 52)
        xnT0 = xntpool.tile([P, CH], fp32, tag="xnt0")
        xnT1 = xntpool.tile([K1, CH], fp32, tag="xnt1")
        for t in range(TPC):
            xt = xpool.tile([P, INPUT_DIM], fp32, tag="xt")
            nc.sync.dma_start(out=xt, in_=x[r0 + t * P : r0 + (t + 1) * P, :])

            st1 = stpool.tile([P, 6], fp32, tag="st1")
            nc.vector.bn_stats(out=st1, in_=xt)
            mv1 = stpool.tile([P, 2], fp32, tag="mv1")
            nc.vector.bn_aggr(out=mv1, in_=st1)

            rstd1 = stpool.tile([P, 1], fp32, tag="rstd1")
            nc.scalar.activation(
                out=rstd1, in_=mv1[:, 1:2], func=AF.Sqrt, bias=eps_t, scale=1.0
            )
            nc.vector.reciprocal(out=rstd1, in_=rstd1)

            xn = xnpool.tile([P, INPUT_DIM], fp32, tag="xn")
            nc.vector.tensor_scalar(
                out=xn,
                in0=xt,
                scalar1=mv1[:, 0:1],
                scalar2=rstd1,
                op0=OP.subtract,
                op1=OP.mult,
            )
            nc.any.tensor_mul(out=xn, in0=xn, in1=g1r)
            nc.any.tensor_add(out=xn, in0=xn, in1=b1r)

            tp0 = smallp.tile([P, P], fp32, tag="smallp")
            nc.tensor.transpose(tp0, xn[:, 0:K0], ident)
            nc.any.tensor_copy(out=xnT0[:, ts(t, P)], in_=tp0)
            tp1 = smallp.tile([P, P], fp32, tag="smallp")
            nc.tensor.transpose(tp1[:K1, :], xn[:, K0:INPUT_DIM], ident)
            nc.any.tensor_copy(out=xnT1[:, ts(t, P)], in_=tp1[:K1, :])

        # q^T chunk: [1536, CH] as 12 x [128, CH]
        qT = qtpool.tile([P, NA, CH], fp32, tag="qt")
        for j in range(NA):
            pq = qpsum.tile([P, CH], fp32, tag="qpsum")
            nc.tensor.matmul(pq, wq0[:, ts(j, P)], xnT0, start=True, stop=False)
            nc.tensor.matmul(pq, wq1[:, ts(j, P)], xnT1, start=False, stop=True)
            nc.any.tensor_copy(out=qT[:, j, :], in_=pq)

        # dots^T chunk: [64, CH]
        pd = dpsum.tile([S, CH], fp32, tag="dpsum")
        for j in range(NA):
            nc.tensor.matmul(
                pd, kT[:, j, :], qT[:, j, :], start=(j == 0), stop=(j == NA - 1)
            )

        # exp (scale folded in); unnormalized attention weights, transposed
        expT = etpool.tile([S, CH], fp32, tag="expT")
        nc.scalar.activation(out=expT, in_=pd, func=AF.Exp, scale=SCALE)

        for t in range(TPC):
            rows = slice(r0 + t * P, r0 + (t + 1) * P)

            # natural-layout w tile + row sums -> normalized w output
            pw = smallp.tile([P, P], fp32, tag="smallp")
            nc.tensor.transpose(pw[:, :S], expT[:, ts(t, P)], ident[:S, :S])
            rsum = stpool.tile([P, 1], fp32, tag="rsum")
            nc.vector.reduce_sum(out=rsum, in_=pw[:, :S], axis=AX.X)
            rcp = stpool.tile([P, 1], fp32, tag="rcp")
            nc.vector.reciprocal(out=rcp, in_=rsum)
            wn = wnpool.tile([P, S], fp32, tag="wn")
            nc.vector.tensor_scalar_mul(wn, pw[:, :S], rcp)
            nc.sync.dma_start(out=out_w[rows, :], in_=wn)

            # s_un = expT_t.T @ slots  (3 x [128, 512] psum tiles)
            ps_list = []
            for nj in range(NN):
                ps = spsum.tile([P, CH], fp32, tag="spsum")
                nc.tensor.matmul(
                    ps,
                    expT[:, ts(t, P)],
                    slots_sb[:, ts(nj, CH)],
                    start=True,
                    stop=True,
                )
                ps_list.append(ps)

            # LN2 stats on unnormalized s
            st2 = stpool.tile([P, NN, 6], fp32, tag="st2")
            for nj in range(NN):
                nc.vector.bn_stats(out=st2[:, nj, :], in_=ps_list[nj])
            mv2 = stpool.tile([P, 2], fp32, tag="mv2")
            nc.vector.bn_aggr(out=mv2, in_=st2)

            # c = rcp * rsqrt(rcp^2 * var_un + eps)  (exact softmax folding)
            rcp2 = stpool.tile([P, 1], fp32, tag="rcp2")
            nc.vector.tensor_mul(out=rcp2, in0=rcp, in1=rcp)
            cmul = stpool.tile([P, 1], fp32, tag="cmul")
            nc.vector.tensor_mul(out=cmul, in0=mv2[:, 1:2], in1=rcp2)
            nc.scalar.activation(
                out=cmul, in_=cmul, func=AF.Sqrt, bias=eps_t, scale=1.0
            )
            nc.vector.reciprocal(out=cmul, in_=cmul)
            nc.vector.tensor_mul(out=cmul, in0=cmul, in1=rcp)

            ot = outpool.tile([P, SLOT_DIM], fp32, tag="ot")
            for nj in range(NN):
                nc.vector.tensor_scalar(
                    out=ot[:, ts(nj, CH)],
                    in0=ps_list[nj],
                    scalar1=mv2[:, 0:1],
                    scalar2=cmul,
                    op0=OP.subtract,
                    op1=OP.mult,
                )
            nc.gpsimd.tensor_mul(out=ot, in0=ot, in1=g2r)
            nc.gpsimd.tensor_add(out=ot, in0=ot, in1=b2r)
            nc.sync.dma_start(out=out_s[rows, :], in_=ot)


def build(I_rows=I_FULL):
    import concourse.bacc as bacc
    import concourse.mybir as mybir
    import concourse.tile as tile

    fp32 = mybir.dt.float32
    nc = bacc.Bacc(
        "TRN2",
        target_bir_lowering=False,
        debug=False,
        enable_asserts=False,
        num_devices=N_CORES,
    )
    aps = {}
    aps["x"] = nc.dram_tensor("x", [I_rows, INPUT_DIM], fp32, kind="ExternalInput").ap()
    aps["slots"] = nc.dram_tensor("slots", [S, SLOT_DIM], fp32, kind="ExternalInput").ap()
    aps["Wq"] = nc.dram_tensor("Wq", [INPUT_DIM, ATT_DIM], fp32, kind="ExternalInput").ap()
    aps["Wk"] = nc.dram_tensor("Wk", [SLOT_DIM, ATT_DIM], fp32, kind="ExternalInput").ap()
    for n, d in (("g1", INPUT_DIM), ("b1", INPUT_DIM), ("g2", SLOT_DIM), ("b2", SLOT_DIM)):
        aps[n] = nc.dram_tensor(n, [d], fp32, kind="ExternalInput").ap()
    aps["out_s"] = nc.dram_tensor(
        "out_s", [I_rows, SLOT_DIM], fp32, kind="ExternalOutput"
    ).ap()
    aps["out_w"] = nc.dram_tensor("out_w", [I_rows, S], fp32, kind="ExternalOutput").ap()

    with tile.TileContext(nc) as tc, ExitStack() as ctx:
        emit(
            tc,
            ctx,
            aps["x"],
            aps["slots"],
            aps["Wq"],
            aps["Wk"],
            aps["g1"],
            aps["b1"],
            aps["g2"],
            aps["b2"],
            aps["out_s"],
            aps["out_w"],
            I_rows,
        )
    nc.compile()
    return nc


_NC_CACHE = {}
TRACE = False
LAST_RESULT = None


def kernel(**inputs):
    global LAST_RESULT
    from concourse.bass_utils import run_bass_kernel_spmd

    x = np.ascontiguousarray(inputs["x"], dtype=np.float32)
    slots = np.ascontiguousarray(inputs["slot_latents"], dtype=np.float32)
    Wq = np.ascontiguousarray(inputs["Wq"], dtype=np.float32)
    Wk = np.ascontiguousarray(inputs["Wk"], dtype=np.float32)
    g1 = np.ascontiguousarray(inputs["g1"], dtype=np.float32)
    b1 = np.ascontiguousarray(inputs["b1"], dtype=np.float32)
    g2 = np.ascontiguousarray(inputs["g2"], dtype=np.float32)
    b2 = np.ascontiguousarray(inputs["b2"], dtype=np.float32)

    bsz, I_rows, _ = x.shape
    assert bsz == N_CORES

    if I_rows not in _NC_CACHE:
        _NC_CACHE[I_rows] = build(I_rows)
    nc = _NC_CACHE[I_rows]

    in_maps = [
        {
            "x": x[b],
            "slots": slots[b],
            "Wq": Wq,
            "Wk": Wk,
            "g1": g1,
            "b1": b1,
            "g2": g2,
            "b2": b2,
        }
        for b in range(N_CORES)
    ]
    kwargs = {}
    if TRACE:
        kwargs = dict(trace=True, trace_cores=[0])
    res = run_bass_kernel_spmd(nc, in_maps, core_ids=list(range(N_CORES)), **kwargs)
    LAST_RESULT = res
    s = np.stack([r["out_s"] for r in res.results])
    w = np.stack([r["out_w"] for r in res.results])
    return s, w


# revision 11
# speedup vs baseline: 17593.8705x; 1.7960x over previous
"""Trainium2 Bass kernel for nn_MixingBlock (slot-attention mixing block).

Math (per batch b):
    xn   = layernorm(x, g1, b1)          # (I, 180)
    q    = xn @ Wq                       # (I, 1536)
    k    = slots @ Wk                    # (64, 1536)
    dots = q @ k.T / sqrt(1536)          # (I, 64)
    w    = softmax(dots, axis=-1)        # (I, 64)   [output]
    s    = layernorm(w @ slots, g2, b2)  # (I, 1536) [output]

Sharding: pure data-parallel over B (B == 8 == n_cores); each core owns one
batch and does no cross-core communication.

Per-core orientation plan (contraction must live on the partition axis):
  - LN1 in natural layout, then PE-transpose xn tiles -> xnT (180 on
    partitions, split 128+52).
  - q^T = Wq_block.T @ xnT  (per 128-wide att block; Wq used in natural
    layout as the stationary operand).
  - dots^T = kT_sub.T-accumulated over 12 att subtiles @ q^T (k^T built
    once at startup from slots^T and Wk).
  - exp without max-subtraction (dots are O(+-6), safe in fp32); the
    1/sqrt(1536) scale is folded into the ACT Exp's input scale.
  - natural-layout w tiles via PE-transpose of exp^T; row sums + normalize
    give the w output.
  - s_un = exp^T.T @ slots with *unnormalized* attention; the softmax
    normalization is folded into LN2's per-row scalars exactly:
      out = (s - mu_s) * rsqrt(var_s + eps) * g2 + b2,  s = r * s_un
          = (s_un - mu_un) * [r * rsqrt(r^2 var_un + eps)] * g2 + b2
    where r = 1/rowsum(exp).
"""

import os
from contextlib import ExitStack

import numpy as np

INPUT_DIM, SLOT_DIM, ATT_DIM = 180, 1536, 1536
B, I_FULL, S = 8, 8192, 64
N_CORES = 8
K0 = 128
K1 = INPUT_DIM - K0  # 52
CH = 512             # rows per chunk
P = 128
TPC = CH // P        # row-tiles per chunk
NA = ATT_DIM // P    # 12 att-dim subtiles
NN = SLOT_DIM // CH  # 3 N-chunks of the 1536-wide outputs
NK = SLOT_DIM // P   # 12 slot-dim subtiles
SCALE = float(ATT_DIM) ** -0.5
EPS = 1e-5


DT_MM = "bf16"  # matmul precision: "f32" | "f32r" | "bf16"


def emit(tc, ctx, x, slots, Wq, Wk, g1, b1, g2, b2, out_s, out_w, I_rows):
    """Emit the Tile program. All tensor args are bass APs."""
    import concourse.bass as bass
    import concourse.mybir as mybir
    from concourse.bass import ts
    from concourse.masks import make_identity

    nc = tc.nc
    fp32 = mybir.dt.float32
    AF = mybir.ActivationFunctionType
    OP = mybir.AluOpType
    AX = mybir.AxisListType

    # matmul-operand dtype handling
    if DT_MM == "bf16":
        mmdt = mybir.dt.bfloat16
        mmcast = lambda ap: ap  # tiles are natively bf16
    elif DT_MM == "f32r":
        mmdt = fp32
        mmcast = lambda ap: ap.bitcast(mybir.dt.float32r)
    else:
        mmdt = fp32
        mmcast = lambda ap: ap

    n_chunks = I_rows // CH
    assert I_rows % CH == 0

    const = ctx.enter_context(tc.tile_pool(name="const", bufs=1))
    wkpool = ctx.enter_context(tc.tile_pool(name="wkstream", bufs=2))
    xpool = ctx.enter_context(tc.tile_pool(name="xin", bufs=3))
    xnpool = ctx.enter_context(tc.tile_pool(name="xn", bufs=3))
    stpool = ctx.enter_context(tc.tile_pool(name="stats", bufs=6))
    xntpool = ctx.enter_context(tc.tile_pool(name="xnt", bufs=2))
    qtpool = ctx.enter_context(tc.tile_pool(name="qt", bufs=2))
    etpool = ctx.enter_context(tc.tile_pool(name="expT", bufs=2))
    wnpool = ctx.enter_context(tc.tile_pool(name="wnat", bufs=3))
    outpool = ctx.enter_context(tc.tile_pool(name="outs", bufs=3))

    # PSUM pools — 8 banks total: 2 + 3 + 1 + 2.
    qpsum = ctx.enter_context(tc.tile_pool(name="qpsum", bufs=2, space="PSUM"))
    spsum = ctx.enter_context(tc.tile_pool(name="spsum", bufs=3, space="PSUM"))
    dpsum = ctx.enter_context(tc.tile_pool(name="dpsum", bufs=1, space="PSUM"))
    smallp = ctx.enter_context(tc.tile_pool(name="smallpsum", bufs=2, space="PSUM"))

    # ---- constants -------------------------------------------------------
    ident = const.tile([P, P], fp32)
    make_identity(nc, ident)

    scratch = ctx.enter_context(tc.tile_pool(name="scratch", bufs=2))

    def load_mm(pool, dram_ap, shape, tag):
        """DMA fp32 DRAM -> SBUF tile of mmdt (converting if needed)."""
        t = pool.tile(shape, mmdt, tag=tag)
        if mmdt == fp32:
            nc.sync.dma_start(out=t, in_=dram_ap)
        else:
            tmp = scratch.tile(shape, fp32, tag="ldscratch")
            nc.sync.dma_start(out=tmp, in_=dram_ap)
            nc.any.tensor_copy(out=t, in_=tmp)
        return t

    wq0 = load_mm(const, Wq[0:K0, :], [K0, ATT_DIM], "wq0")
    wq1 = load_mm(const, Wq[K0:INPUT_DIM, :], [K1, ATT_DIM], "wq1")

    slots_f32 = const.tile([S, SLOT_DIM], fp32)
    nc.sync.dma_start(out=slots_f32, in_=slots)
    if mmdt == fp32:
        slots_mm = slots_f32
    else:
        slots_mm = const.tile([S, SLOT_DIM], mmdt)
        nc.any.tensor_copy(out=slots_mm, in_=slots_f32)

    def bcast(ap):
        return bass.AP(tensor=ap.tensor, offset=ap.offset, ap=[[0, P], *ap.ap])

    g1r = const.tile([P, INPUT_DIM], fp32)
    nc.gpsimd.dma_start(out=g1r, in_=bcast(g1))
    b1r = const.tile([P, INPUT_DIM], fp32)
    nc.gpsimd.dma_start(out=b1r, in_=bcast(b1))
    g2r = const.tile([P, SLOT_DIM], fp32)
    nc.gpsimd.dma_start(out=g2r, in_=bcast(g2))
    b2r = const.tile([P, SLOT_DIM], fp32)
    nc.gpsimd.dma_start(out=b2r, in_=bcast(b2))

    eps_t = const.tile([P, 1], fp32)
    nc.vector.memset(eps_t, EPS)

    # ---- slots^T: [64, 1536] -> 12 x [128, 64] ---------------------------
    slotsT = const.tile([P, NK, S], mmdt)
    for j in range(NK):
        pt = smallp.tile([P, P], fp32, tag="smallp")
        nc.tensor.transpose(pt[:, :S], slots_f32[:, ts(j, P)], ident[:S, :S])
        nc.any.tensor_copy(out=slotsT[:, j, :], in_=pt[:, :S])

    # ---- k^T = (slots @ Wk)^T : 12 x [128, 64] ---------------------------
    # kT[a, s] = sum_d Wk[d, a] * slotsT[d, s]
    kT = const.tile([P, NA, S], mmdt)
    for j in range(NA):
        wkblk = load_mm(
            wkpool,
            Wk.rearrange("(ko ki) a -> ki ko a", ki=P)[:, :, ts(j, P)],
            [P, NK, P],
            "wkblk",
        )
        pk = smallp.tile([P, P], fp32, tag="smallp")
        for ksub in range(NK):
            nc.tensor.matmul(
                pk[:, :S],
                mmcast(wkblk[:, ksub, :]),
                mmcast(slotsT[:, ksub, :]),
                start=(ksub == 0),
                stop=(ksub == NK - 1),
            )
        nc.any.tensor_copy(out=kT[:, j, :], in_=pk[:, :S])

    # ---- main loop over row chunks ---------------------------------------
    for c in range(n_chunks):
        r0 = c * CH

        # LN1 + transpose -> xnT (180 x CH, split 128 + 52)
        xnT0 = xntpool.tile([P, CH], mmdt, tag="xnt0")
        xnT1 = xntpool.tile([K1, CH], mmdt, tag="xnt1")
        for t in range(TPC):
            xt = xpool.tile([P, INPUT_DIM], fp32, tag="xt")
            nc.sync.dma_start(out=xt, in_=x[r0 + t * P : r0 + (t + 1) * P, :])

            st1 = stpool.tile([P, 6], fp32, tag="st1")
            nc.vector.bn_stats(out=st1, in_=xt)
            mv1 = stpool.tile([P, 2], fp32, tag="mv1")
            nc.vector.bn_aggr(out=mv1, in_=st1)

            rstd1 = stpool.tile([P, 1], fp32, tag="rstd1")
            nc.scalar.activation(
                out=rstd1, in_=mv1[:, 1:2], func=AF.Sqrt, bias=eps_t, scale=1.0
            )
            nc.vector.reciprocal(out=rstd1, in_=rstd1)

            xn = xnpool.tile([P, INPUT_DIM], fp32, tag="xn")
            nc.vector.tensor_scalar(
                out=xn,
                in0=xt,
                scalar1=mv1[:, 0:1],
                scalar2=rstd1,
                op0=OP.subtract,
                op1=OP.mult,
            )
            nc.any.tensor_mul(out=xn, in0=xn, in1=g1r)
            nc.any.tensor_add(out=xn, in0=xn, in1=b1r)

            tp0 = smallp.tile([P, P], fp32, tag="smallp")
            nc.tensor.transpose(tp0, xn[:, 0:K0], ident)
            nc.any.tensor_copy(out=xnT0[:, ts(t, P)], in_=tp0)
            tp1 = smallp.tile([P, P], fp32, tag="smallp")
            nc.tensor.transpose(tp1[:K1, :], xn[:, K0:INPUT_DIM], ident)
            nc.any.tensor_copy(out=xnT1[:, ts(t, P)], in_=tp1[:K1, :])

        # q^T chunk: [1536, CH] as 12 x [128, CH]
        qT = qtpool.tile([P, NA, CH], mmdt, tag="qt")
        for j in range(NA):
            pq = qpsum.tile([P, CH], fp32, tag="qpsum")
            nc.tensor.matmul(
                pq, mmcast(wq0[:, ts(j, P)]), mmcast(xnT0), start=True, stop=False
            )
            nc.tensor.matmul(
                pq, mmcast(wq1[:, ts(j, P)]), mmcast(xnT1), start=False, stop=True
            )
            nc.any.tensor_copy(out=qT[:, j, :], in_=pq)

        # dots^T chunk: [64, CH]
        pd = dpsum.tile([S, CH], fp32, tag="dpsum")
        for j in range(NA):
            nc.tensor.matmul(
                pd,
                mmcast(kT[:, j, :]),
                mmcast(qT[:, j, :]),
                start=(j == 0),
                stop=(j == NA - 1),
            )

        # exp (scale folded in); unnormalized attention weights, transposed
        expT = etpool.tile([S, CH], fp32, tag="expT")
        nc.scalar.activation(out=expT, in_=pd, func=AF.Exp, scale=SCALE)
        if mmdt == fp32:
            expT_mm = expT
        else:
            expT_mm = etpool.tile([S, CH], mmdt, tag="expTmm")
            nc.any.tensor_copy(out=expT_mm, in_=expT)

        for t in range(TPC):
            rows = slice(r0 + t * P, r0 + (t + 1) * P)

            # natural-layout w tile + row sums -> normalized w output
            pw = smallp.tile([P, P], fp32, tag="smallp")
            nc.tensor.transpose(pw[:, :S], expT[:, ts(t, P)], ident[:S, :S])
            rsum = stpool.tile([P, 1], fp32, tag="rsum")
            nc.vector.reduce_sum(out=rsum, in_=pw[:, :S], axis=AX.X)
            rcp = stpool.tile([P, 1], fp32, tag="rcp")
            nc.vector.reciprocal(out=rcp, in_=rsum)
            wn = wnpool.tile([P, S], fp32, tag="wn")
            nc.vector.tensor_scalar_mul(wn, pw[:, :S], rcp)
            nc.sync.dma_start(out=out_w[rows, :], in_=wn)

            # s_un = expT_t.T @ slots  (3 x [128, 512] psum tiles)
            ps_list = []
            for nj in range(NN):
                ps = spsum.tile([P, CH], fp32, tag="spsum")
                nc.tensor.matmul(
                    ps,
                    mmcast(expT_mm[:, ts(t, P)]),
                    mmcast(slots_mm[:, ts(nj, CH)]),
                    start=True,
                    stop=True,
                )
                ps_list.append(ps)

            # LN2 stats on unnormalized s
            st2 = stpool.tile([P, NN, 6], fp32, tag="st2")
            for nj in range(NN):
                nc.vector.bn_stats(out=st2[:, nj, :], in_=ps_list[nj])
            mv2 = stpool.tile([P, 2], fp32, tag="mv2")
            nc.vector.bn_aggr(out=mv2, in_=st2)

            # c = rcp * rsqrt(rcp^2 * var_un + eps)  (exact softmax folding)
            rcp2 = stpool.tile([P, 1], fp32, tag="rcp2")
            nc.vector.tensor_mul(out=rcp2, in0=rcp, in1=rcp)
            cmul = stpool.tile([P, 1], fp32, tag="cmul")
            nc.vector.tensor_mul(out=cmul, in0=mv2[:, 1:2], in1=rcp2)
            nc.scalar.activation(
                out=cmul, in_=cmul, func=AF.Sqrt, bias=eps_t, scale=1.0
            )
            nc.vector.reciprocal(out=cmul, in_=cmul)
            nc.vector.tensor_mul(out=cmul, in0=cmul, in1=rcp)

            ot = outpool.tile([P, SLOT_DIM], fp32, tag="ot")
            for nj in range(NN):
                nc.vector.tensor_scalar(
                    out=ot[:, ts(nj, CH)],
                    in0=ps_list[nj],
                    scalar1=mv2[:, 0:1],
                    scalar2=cmul,
                    op0=OP.subtract,
                    op1=OP.mult,
                )
            nc.gpsimd.tensor_mul(out=ot, in0=ot, in1=g2r)
            nc.gpsimd.tensor_add(out=ot, in0=ot, in1=b2r)
            nc.sync.dma_start(out=out_s[rows, :], in_=ot)


def build(I_rows=I_FULL):
    import concourse.bacc as bacc
    import concourse.mybir as mybir
    import concourse.tile as tile

    fp32 = mybir.dt.float32
    nc = bacc.Bacc(
        "TRN2",
        target_bir_lowering=False,
        debug=False,
        enable_asserts=False,
        num_devices=N_CORES,
    )
    aps = {}
    aps["x"] = nc.dram_tensor("x", [I_rows, INPUT_DIM], fp32, kind="ExternalInput").ap()
    aps["slots"] = nc.dram_tensor("slots", [S, SLOT_DIM], fp32, kind="ExternalInput").ap()
    aps["Wq"] = nc.dram_tensor("Wq", [INPUT_DIM, ATT_DIM], fp32, kind="ExternalInput").ap()
    aps["Wk"] = nc.dram_tensor("Wk", [SLOT_DIM, ATT_DIM], fp32, kind="ExternalInput").ap()
    for n, d in (("g1", INPUT_DIM), ("b1", INPUT_DIM), ("g2", SLOT_DIM), ("b2", SLOT_DIM)):
        aps[n] = nc.dram_tensor(n, [d], fp32, kind="ExternalInput").ap()
    aps["out_s"] = nc.dram_tensor(
        "out_s", [I_rows, SLOT_DIM], fp32, kind="ExternalOutput"
    ).ap()
    aps["out_w"] = nc.dram_tensor("out_w", [I_rows, S], fp32, kind="ExternalOutput").ap()

    with tile.TileContext(nc) as tc, ExitStack() as ctx:
        emit(
            tc,
            ctx,
            aps["x"],
            aps["slots"],
            aps["Wq"],
            aps["Wk"],
            aps["g1"],
            aps["b1"],
            aps["g2"],
            aps["b2"],
            aps["out_s"],
            aps["out_w"],
            I_rows,
        )
    nc.compile()
    return nc


_NC_CACHE = {}
TRACE = False
LAST_RESULT = None


def kernel(**inputs):
    global LAST_RESULT
    from concourse.bass_utils import run_bass_kernel_spmd

    x = np.ascontiguousarray(inputs["x"], dtype=np.float32)
    slots = np.ascontiguousarray(inputs["slot_latents"], dtype=np.float32)
    Wq = np.ascontiguousarray(inputs["Wq"], dtype=np.float32)
    Wk = np.ascontiguousarray(inputs["Wk"], dtype=np.float32)
    g1 = np.ascontiguousarray(inputs["g1"], dtype=np.float32)
    b1 = np.ascontiguousarray(inputs["b1"], dtype=np.float32)
    g2 = np.ascontiguousarray(inputs["g2"], dtype=np.float32)
    b2 = np.ascontiguousarray(inputs["b2"], dtype=np.float32)

    bsz, I_rows, _ = x.shape
    assert bsz == N_CORES

    if I_rows not in _NC_CACHE:
        _NC_CACHE[I_rows] = build(I_rows)
    nc = _NC_CACHE[I_rows]

    in_maps = [
        {
            "x": x[b],
            "slots": slots[b],
            "Wq": Wq,
            "Wk": Wk,
            "g1": g1,
            "b1": b1,
            "g2": g2,
            "b2": b2,
        }
        for b in range(N_CORES)
    ]
    kwargs = {}
    if TRACE:
        kwargs = dict(trace=True, trace_cores=[0])
    res = run_bass_kernel_spmd(nc, in_maps, core_ids=list(range(N_CORES)), **kwargs)
    LAST_RESULT = res
    s = np.stack([r["out_s"] for r in res.results])
    w = np.stack([r["out_w"] for r in res.results])
    return s, w
